# revision 9
# baseline (speedup 1.0000x reference)
"""Plackett-Luce listwise loss kernel for Trainium2 (Bass/Tile), 8-core data parallel.

Algorithm (per row of 32 items):
  loss_row = sum_k log(T_k) - sum_i s_i*valid_i, where T_k are the suffix sums
  of exp(s) over items sorted by (rank, position) (stable sort, padded last).
  Final: mean over rows with n>=2 of loss_row/n.

Device strategy: pack key = rank_eff*2^19 + position*2^14 + (s+8)*2^10 into one
fp32 (rank_eff = rank + 64*mask, so padded rows sort last), sort each row's 32
keys DESCENDING with a Batcher odd-even merge network (15 min/max stages on
strided access patterns), then decode the quantized score from the low bits
(s error <= 2^-10, final loss rel err ~1e-7), exp on ScalarE, segmented prefix
scan for the suffix sums, log on ScalarE, and per-row reductions. Each core
reduces its 32768 rows to a [128, 2] partial (weighted loss sum, valid-row
count); the host sums partials and divides.
"""

import sys

for _p in ("/opt/trn_rl_repo", "/root/.axon_site/_ro/trn_rl_repo"):
    if _p not in sys.path:
        sys.path.insert(0, _p)

import numpy as np

P = 128
N = 32
NCORES = 8
B = 262144
B_CORE = B // NCORES  # 32768
J = 32                # row-segments per partition per supertile
F = J * N             # free elements per supertile tile
ROWS_SUP = P * J      # rows per supertile
SUP = B_CORE // ROWS_SUP

# Batcher odd-even merge sort, n=32, descending.
# (k, offset, per-segment pattern [[step,count],...], needs_precopy)
SORT_STAGES = [
    (1, 0, [[2, 16]], False),
    (2, 0, [[4, 8], [1, 2]], False),
    (1, 1, [[4, 8]], True),
    (4, 0, [[8, 4], [1, 4]], False),
    (2, 2, [[8, 4], [1, 2]], True),
    (1, 1, [[8, 4], [2, 3]], True),
    (8, 0, [[16, 2], [1, 8]], False),
    (4, 4, [[16, 2], [1, 4]], True),
    (2, 2, [[16, 2], [4, 3], [1, 2]], True),
    (1, 1, [[16, 2], [2, 7]], True),
    (16, 0, [[1, 16]], False),
    (8, 8, [[1, 8]], True),
    (4, 4, [[8, 3], [1, 4]], True),
    (2, 2, [[4, 7], [1, 2]], True),
    (1, 1, [[2, 15]], True),
]

SC_POS = float(2 ** 14)   # position scale in the packed key
SC_RANK = float(2 ** 19)  # rank scale
SC_S = float(2 ** 10)     # score scale
MASK_BUMP = float(2 ** 25)  # added to the key of padded items
# Valid keys < 2^24 (rank<32); padded keys >= 2^25 - 2^13. Threshold between:
INVALID_THRESH = float(2 ** 24 + 2 ** 23)
RND = float(2 ** 23)      # fp32 round-to-nearest-integer magic constant


def _pattern_ap(bass_mod, tile_ap, off, dims, j):
    """AP over a [P, j*32] tile selecting `dims` within each 32-item segment."""
    base = tile_ap
    pdim = base.ap[0]
    if dims[0][0] * dims[0][1] == N:
        free = [[dims[0][0], dims[0][1] * j]] + [list(d) for d in dims[1:]]
    else:
        free = [[N, j]] + [list(d) for d in dims]
    return bass_mod.AP(tensor=base.tensor, offset=base.offset + off, ap=[list(pdim)] + free)


def build_program(b_core=B_CORE, j=J):
    import concourse.bass as bass
    import concourse.bacc as bacc
    import concourse.tile as tile
    from concourse import mybir

    f = j * N
    rows_sup = P * j
    sup_count = b_core // rows_sup
    assert b_core % rows_sup == 0

    # Bacc (not raw Bass): its compile() runs generate_event_semaphores, which
    # splits multi-sem waits that TRN2 compute instructions can't encode.
    nc = bacc.Bacc("TRN2")
    s_d = nc.dram_tensor("scores", [b_core, N], mybir.dt.float32, kind="ExternalInput")
    r_d = nc.dram_tensor("ranks32", [b_core, 2 * N], mybir.dt.int32, kind="ExternalInput")
    m_d = nc.dram_tensor("mask8", [b_core, N], mybir.dt.uint8, kind="ExternalInput")
    o_d = nc.dram_tensor("partial", [P, 2], mybir.dt.float32, kind="ExternalOutput")

    op = mybir.AluOpType
    act = mybir.ActivationFunctionType

    with tile.TileContext(nc) as tc:
        with (
            tc.tile_pool(name="singles", bufs=1) as singles,
            tc.tile_pool(name="stream", bufs=2) as stream,
        ):
            # constants
            iota14 = singles.tile([P, f], mybir.dt.int32)
            nc.gpsimd.iota(iota14[:], pattern=[[0, j], [int(SC_POS), N]], base=0,
                           channel_multiplier=0)
            gate = singles.tile([P, f], mybir.dt.float32)
            nc.vector.memset(gate[:], 1.0)
            g3 = gate[:].rearrange("p (j n) -> p j n", n=N)
            nc.vector.memset(g3[:, :, 0:1], 0.0)


            # per-row stats accumulated across supertiles
            js = j * sup_count
            lsum_all = singles.tile([P, js], mybir.dt.float32)
            svr_all = singles.tile([P, js], mybir.dt.float32)
            nm_all = singles.tile([P, js], mybir.dt.float32)

            for sup in range(sup_count):
                r0 = sup * rows_sup
                s_t = stream.tile([P, f], mybir.dt.float32)
                nc.sync.dma_start(
                    out=s_t[:],
                    in_=s_d[r0:r0 + rows_sup, :].rearrange("(p j) n -> p (j n)", p=P))
                r_t = stream.tile([P, 2 * f], mybir.dt.int32)
                nc.sync.dma_start(
                    out=r_t[:],
                    in_=r_d[r0:r0 + rows_sup, :].rearrange("(p j) n -> p (j n)", p=P))
                m_t = stream.tile([P, f], mybir.dt.uint8)
                nc.sync.dma_start(
                    out=m_t[:],
                    in_=m_d[r0:r0 + rows_sup, :].rearrange("(p j) n -> p (j n)", p=P))

                # ---- pack V = rank*2^19 + mask*2^25 + i*2^14 + s*2^10
                # (all on DVE, not ACT: the ACT sync struct supports a single
                # wait command, so ACT ops must not read DMA-produced tiles)
                sp = stream.tile([P, f], mybir.dt.float32)
                nc.vector.tensor_single_scalar(out=sp[:], in_=s_t[:], scalar=SC_S,
                                               op=op.mult)
                r_lo = r_t[:].rearrange("p (f two) -> p f two", two=2)[:, :, 0]
                q_t = stream.tile([P, f], mybir.dt.float32)
                nc.vector.scalar_tensor_tensor(
                    out=q_t[:], in0=r_lo, scalar=SC_RANK, in1=iota14[:],
                    op0=op.mult, op1=op.add)
                w2 = stream.tile([P, f], mybir.dt.float32)
                nc.vector.scalar_tensor_tensor(
                    out=w2[:], in0=m_t[:], scalar=MASK_BUMP, in1=sp[:],
                    op0=op.mult, op1=op.add)
                v_a = stream.tile([P, f], mybir.dt.float32)
                nc.vector.tensor_tensor(out=v_a[:], in0=q_t[:], in1=w2[:], op=op.add)

                # ---- per-row masked score sum and mask count (pre-sort)
                sm = stream.tile([P, f], mybir.dt.float32)
                nc.vector.scalar_tensor_tensor(
                    out=sm[:], in0=m_t[:], scalar=0.0, in1=s_t[:],
                    op0=op.is_equal, op1=op.mult)
                nc.vector.tensor_reduce(
                    out=svr_all[:, sup * j:(sup + 1) * j],
                    in_=sm[:].rearrange("p (j n) -> p j n", n=N),
                    axis=mybir.AxisListType.X, op=op.add)
                nc.vector.tensor_reduce(
                    out=nm_all[:, sup * j:(sup + 1) * j],
                    in_=m_t[:].rearrange("p (j n) -> p j n", n=N),
                    axis=mybir.AxisListType.X, op=op.add)

                # ---- Batcher descending sort (ping-pong v_a <-> v_b)
                v_b = stream.tile([P, f], mybir.dt.float32)
                cur, oth = v_a, v_b
                for (k, off, dims, precopy) in SORT_STAGES:
                    if precopy:
                        nc.vector.tensor_copy(out=oth[:], in_=cur[:])
                    lo_i = _pattern_ap(bass, cur[:], off, dims, j)
                    hi_i = _pattern_ap(bass, cur[:], off + k, dims, j)
                    lo_o = _pattern_ap(bass, oth[:], off, dims, j)
                    hi_o = _pattern_ap(bass, oth[:], off + k, dims, j)
                    nc.vector.tensor_tensor(out=lo_o, in0=lo_i, in1=hi_i, op=op.max)
                    nc.vector.tensor_tensor(out=hi_o, in0=lo_i, in1=hi_i, op=op.min)
                    cur, oth = oth, cur
                v_s = cur

                # ---- decode: u = V mod 2^14 (centered, in (-2^13, 2^13)) via
                # the +2^23 round-to-nearest trick (no mod/convert ISA needed)
                t1 = stream.tile([P, f], mybir.dt.float32)
                nc.vector.tensor_scalar(out=t1[:], in0=v_s[:], scalar1=1.0 / SC_POS,
                                        scalar2=RND, op0=op.mult, op1=op.add)
                wf = stream.tile([P, f], mybir.dt.float32)
                nc.vector.tensor_single_scalar(out=wf[:], in_=t1[:], scalar=-RND,
                                               op=op.add)
                u_t = stream.tile([P, f], mybir.dt.float32)
                nc.vector.scalar_tensor_tensor(
                    out=u_t[:], in0=wf[:], scalar=-SC_POS, in1=v_s[:],
                    op0=op.mult, op1=op.add)
                e_t = stream.tile([P, f], mybir.dt.float32)
                nc.scalar.activation(out=e_t[:], in_=u_t[:], func=act.Exp,
                                     scale=1.0 / SC_S)
                ez = stream.tile([P, f], mybir.dt.float32)
                nc.vector.scalar_tensor_tensor(
                    out=ez[:], in0=v_s[:], scalar=INVALID_THRESH, in1=e_t[:],
                    op0=op.is_lt, op1=op.mult)
                t_t = stream.tile([P, f], mybir.dt.float32)
                nc.vector.tensor_tensor_scan(
                    out=t_t[:], data0=gate[:], data1=ez[:], initial=0.0,
                    op0=op.mult, op1=op.add)
                nc.vector.scalar_tensor_tensor(
                    out=t_t[:], in0=v_s[:], scalar=INVALID_THRESH, in1=t_t[:],
                    op0=op.is_ge, op1=op.add)
                lg = stream.tile([P, f], mybir.dt.float32)
                nc.scalar.activation(out=lg[:], in_=t_t[:], func=act.Ln)
                nc.vector.tensor_reduce(
                    out=lsum_all[:, sup * j:(sup + 1) * j],
                    in_=lg[:].rearrange("p (j n) -> p j n", n=N),
                    axis=mybir.AxisListType.X, op=op.add)

            # ---- epilogue: per-row weighting, partition-level partials
            n_t = singles.tile([P, js], mybir.dt.float32)
            nc.vector.tensor_scalar(out=n_t[:], in0=nm_all[:], scalar1=-1.0,
                                    scalar2=float(N), op0=op.mult, op1=op.add)
            pr0 = singles.tile([P, js], mybir.dt.float32)
            nc.vector.tensor_sub(pr0[:], lsum_all[:], svr_all[:])
            nmx = singles.tile([P, js], mybir.dt.float32)
            nc.vector.tensor_scalar_max(nmx[:], n_t[:], 1.0)
            wrec = singles.tile([P, js], mybir.dt.float32)
            nc.vector.reciprocal(wrec[:], nmx[:])
            use = singles.tile([P, js], mybir.dt.float32)
            nc.vector.tensor_single_scalar(out=use[:], in_=n_t[:], scalar=2.0,
                                           op=op.is_ge)
            w3 = singles.tile([P, js], mybir.dt.float32)
            nc.vector.tensor_tensor(out=w3[:], in0=wrec[:], in1=use[:], op=op.mult)
            pr = singles.tile([P, js], mybir.dt.float32)
            nc.vector.tensor_tensor(out=pr[:], in0=pr0[:], in1=w3[:], op=op.mult)

            out_t = singles.tile([P, 2], mybir.dt.float32)
            nc.vector.tensor_reduce(out=out_t[:, 0:1], in_=pr[:],
                                    axis=mybir.AxisListType.X, op=op.add)
            nc.vector.tensor_reduce(out=out_t[:, 1:2], in_=use[:],
                                    axis=mybir.AxisListType.X, op=op.add)
            nc.sync.dma_start(out=o_d[:], in_=out_t[:])

    nc.finalize()  # run Bacc compile passes (wait splitting, reg alloc)
    return nc


_CACHED = {}


def _get_program():
    if "nc" not in _CACHED:
        _CACHED["nc"] = build_program()
    return _CACHED["nc"]


def _run(scores, ranks, mask, **run_kwargs):
    from concourse.bass_utils import run_bass_kernel_spmd

    nc = _get_program()
    scores = np.ascontiguousarray(np.asarray(scores, dtype=np.float32))
    ranks = np.ascontiguousarray(np.asarray(ranks, dtype=np.int64))
    mask = np.ascontiguousarray(np.asarray(mask))

    in_maps = []
    for c in range(NCORES):
        lo, hi = c * B_CORE, (c + 1) * B_CORE
        in_maps.append({
            "scores": scores[lo:hi],
            "ranks32": ranks[lo:hi].view(np.int32).reshape(B_CORE, 2 * N),
            "mask8": mask[lo:hi].astype(np.uint8),
        })
    res = run_bass_kernel_spmd(nc, in_maps, core_ids=list(range(NCORES)), **run_kwargs)
    partials = np.stack([r["partial"] for r in res.results])  # [8, 128, 2]
    loss_sum = partials[:, :, 0].sum(dtype=np.float64)
    cnt = partials[:, :, 1].sum(dtype=np.float64)
    out = np.float32(loss_sum / max(cnt, 1.0))
    return out, res


def kernel(scores, ranks, mask):
    out, _ = _run(scores, ranks, mask)
    return np.asarray(out, dtype=np.float32)


# revision 15
# speedup vs baseline: 1.1875x; 1.1875x over previous
"""Plackett-Luce listwise loss kernel for Trainium2 (Bass/Tile), 8-core data parallel.

Algorithm (per row of 32 items):
  loss_row = sum_k log(T_k) - sum_i s_i*valid_i, where T_k are the suffix sums
  of exp(s) over items sorted by (rank, position) (stable sort, padded last).
  Final: mean over rows with n>=2 of loss_row/n.

Device strategy: pack key = rank_eff*2^19 + position*2^14 + (s+8)*2^10 into one
fp32 (rank_eff = rank + 64*mask, so padded rows sort last), sort each row's 32
keys DESCENDING with a Batcher odd-even merge network (15 min/max stages on
strided access patterns), then decode the quantized score from the low bits
(s error <= 2^-10, final loss rel err ~1e-7), exp on ScalarE, segmented prefix
scan for the suffix sums, log on ScalarE, and per-row reductions. Each core
reduces its 32768 rows to a [128, 2] partial (weighted loss sum, valid-row
count); the host sums partials and divides.
"""

import sys

for _p in ("/opt/trn_rl_repo", "/root/.axon_site/_ro/trn_rl_repo"):
    if _p not in sys.path:
        sys.path.insert(0, _p)

import numpy as np

P = 128
N = 32
NCORES = 8
B = 262144
B_CORE = B // NCORES  # 32768
J = 32                # row-segments per partition per supertile
F = J * N             # free elements per supertile tile
ROWS_SUP = P * J      # rows per supertile
SUP = B_CORE // ROWS_SUP

# Batcher odd-even merge sort, n=32, descending.
# (k, offset, per-segment pattern [[step,count],...], needs_precopy)
SORT_STAGES = [
    (1, 0, [[2, 16]], False),
    (2, 0, [[4, 8], [1, 2]], False),
    (1, 1, [[4, 8]], True),
    (4, 0, [[8, 4], [1, 4]], False),
    (2, 2, [[8, 4], [1, 2]], True),
    (1, 1, [[8, 4], [2, 3]], True),
    (8, 0, [[16, 2], [1, 8]], False),
    (4, 4, [[16, 2], [1, 4]], True),
    (2, 2, [[16, 2], [4, 3], [1, 2]], True),
    (1, 1, [[16, 2], [2, 7]], True),
    (16, 0, [[1, 16]], False),
    (8, 8, [[1, 8]], True),
    (4, 4, [[8, 3], [1, 4]], True),
    (2, 2, [[4, 7], [1, 2]], True),
    (1, 1, [[2, 15]], True),
]

SC_POS = float(2 ** 14)   # position scale in the packed key
SC_RANK = float(2 ** 19)  # rank scale
SC_S = float(2 ** 10)     # score scale
MASK_BUMP = float(2 ** 25)  # added to the key of padded items
# Valid keys < 2^24 (rank<32); padded keys >= 2^25 - 2^13. Threshold between:
INVALID_THRESH = float(2 ** 24 + 2 ** 23)
RND = float(2 ** 23)      # fp32 round-to-nearest-integer magic constant

# Supertiles whose sort network runs on GPSIMD instead of DVE. Empty: plain
# TensorTensor is not a legal Pool-engine opcode on NeuronCore v3.
GPSIMD_SORT_SUPS = ()


def _pattern_ap(bass_mod, tile_ap, off, dims, j):
    """AP over a [P, j*32] tile selecting `dims` within each 32-item segment."""
    base = tile_ap
    pdim = base.ap[0]
    if dims[0][0] * dims[0][1] == N:
        free = [[dims[0][0], dims[0][1] * j]] + [list(d) for d in dims[1:]]
    else:
        free = [[N, j]] + [list(d) for d in dims]
    return bass_mod.AP(tensor=base.tensor, offset=base.offset + off, ap=[list(pdim)] + free)


def build_program(b_core=B_CORE, j=J):
    import concourse.bass as bass
    import concourse.bacc as bacc
    import concourse.tile as tile
    from concourse import mybir

    f = j * N
    rows_sup = P * j
    sup_count = b_core // rows_sup
    assert b_core % rows_sup == 0

    # Bacc (not raw Bass): its compile() runs generate_event_semaphores, which
    # splits multi-sem waits that TRN2 compute instructions can't encode.
    nc = bacc.Bacc("TRN2")
    s_d = nc.dram_tensor("scores", [b_core, N], mybir.dt.float32, kind="ExternalInput")
    r_d = nc.dram_tensor("ranks32", [b_core, 2 * N], mybir.dt.int32, kind="ExternalInput")
    m_d = nc.dram_tensor("mask8", [b_core, N], mybir.dt.uint8, kind="ExternalInput")
    o_d = nc.dram_tensor("partial", [P, 2], mybir.dt.float32, kind="ExternalOutput")

    op = mybir.AluOpType
    act = mybir.ActivationFunctionType

    with tile.TileContext(nc) as tc:
        with (
            tc.tile_pool(name="singles", bufs=1) as singles,
            tc.tile_pool(name="stream", bufs=2) as stream,
        ):
            # constants
            iota14 = singles.tile([P, f], mybir.dt.int32)
            nc.gpsimd.iota(iota14[:], pattern=[[0, j], [int(SC_POS), N]], base=0,
                           channel_multiplier=0)
            gate = singles.tile([P, f], mybir.dt.float32)
            nc.vector.memset(gate[:], 1.0)
            g3 = gate[:].rearrange("p (j n) -> p j n", n=N)
            nc.vector.memset(g3[:, :, 0:1], 0.0)
            c_rnd = singles.tile([P, 1], mybir.dt.float32)
            nc.vector.memset(c_rnd[:], RND)
            c_nrnd = singles.tile([P, 1], mybir.dt.float32)
            nc.vector.memset(c_nrnd[:], -RND)


            # per-row stats accumulated across supertiles
            js = j * sup_count
            lsum_all = singles.tile([P, js], mybir.dt.float32)
            svr_all = singles.tile([P, js], mybir.dt.float32)
            nm_all = singles.tile([P, js], mybir.dt.float32)

            for sup in range(sup_count):
                r0 = sup * rows_sup
                s_t = stream.tile([P, f], mybir.dt.float32)
                nc.sync.dma_start(
                    out=s_t[:],
                    in_=s_d[r0:r0 + rows_sup, :].rearrange("(p j) n -> p (j n)", p=P))
                r_t = stream.tile([P, 2 * f], mybir.dt.int32)
                nc.sync.dma_start(
                    out=r_t[:],
                    in_=r_d[r0:r0 + rows_sup, :].rearrange("(p j) n -> p (j n)", p=P))
                m_t = stream.tile([P, f], mybir.dt.uint8)
                nc.sync.dma_start(
                    out=m_t[:],
                    in_=m_d[r0:r0 + rows_sup, :].rearrange("(p j) n -> p (j n)", p=P))

                # ---- pack V = rank*2^19 + mask*2^25 + i*2^14 + s*2^10
                # chained STT ops on DVE (ACT can't: its sync struct supports a
                # single wait command, so ACT must not read DMA tiles directly)
                r_lo = r_t[:].rearrange("p (f two) -> p f two", two=2)[:, :, 0]
                q_t = stream.tile([P, f], mybir.dt.float32)
                nc.vector.scalar_tensor_tensor(
                    out=q_t[:], in0=r_lo, scalar=SC_RANK, in1=iota14[:],
                    op0=op.mult, op1=op.add)
                w2 = stream.tile([P, f], mybir.dt.float32)
                nc.vector.scalar_tensor_tensor(
                    out=w2[:], in0=m_t[:], scalar=MASK_BUMP, in1=q_t[:],
                    op0=op.mult, op1=op.add)
                v_a = stream.tile([P, f], mybir.dt.float32)
                nc.vector.scalar_tensor_tensor(
                    out=v_a[:], in0=s_t[:], scalar=SC_S, in1=w2[:],
                    op0=op.mult, op1=op.add)

                # ---- per-row masked score sum and mask count (pre-sort)
                sm = stream.tile([P, f], mybir.dt.float32)
                nc.vector.scalar_tensor_tensor(
                    out=sm[:], in0=m_t[:], scalar=0.0, in1=s_t[:],
                    op0=op.is_equal, op1=op.mult)
                nc.vector.tensor_reduce(
                    out=svr_all[:, sup * j:(sup + 1) * j],
                    in_=sm[:].rearrange("p (j n) -> p j n", n=N),
                    axis=mybir.AxisListType.X, op=op.add)
                nc.vector.tensor_reduce(
                    out=nm_all[:, sup * j:(sup + 1) * j],
                    in_=m_t[:].rearrange("p (j n) -> p j n", n=N),
                    axis=mybir.AxisListType.X, op=op.add)

                # ---- Batcher descending sort (ping-pong v_a <-> v_b)
                # min/max on DVE or GPSIMD (supertile round-robin balance);
                # precopies (for stages with unpaired lanes) on the idle ACT.
                sort_eng = nc.gpsimd if (sup % 8) in GPSIMD_SORT_SUPS else nc.vector
                v_b = stream.tile([P, f], mybir.dt.float32)
                cur, oth = v_a, v_b
                for (k, off, dims, precopy) in SORT_STAGES:
                    if precopy:
                        nc.scalar.copy(out=oth[:], in_=cur[:])
                    lo_i = _pattern_ap(bass, cur[:], off, dims, j)
                    hi_i = _pattern_ap(bass, cur[:], off + k, dims, j)
                    lo_o = _pattern_ap(bass, oth[:], off, dims, j)
                    hi_o = _pattern_ap(bass, oth[:], off + k, dims, j)
                    sort_eng.tensor_tensor(out=lo_o, in0=lo_i, in1=hi_i, op=op.max)
                    sort_eng.tensor_tensor(out=hi_o, in0=lo_i, in1=hi_i, op=op.min)
                    cur, oth = oth, cur
                v_s = cur

                # ---- decode: u = V mod 2^14 (centered, in (-2^13, 2^13)) via
                # the +2^23 round-to-nearest trick (no mod/convert ISA needed);
                # the two single-src affine steps ride on the idle ACT engine
                t1 = stream.tile([P, f], mybir.dt.float32)
                nc.scalar.activation(out=t1[:], in_=v_s[:], func=act.Identity,
                                     bias=c_rnd[:], scale=1.0 / SC_POS)
                wf = stream.tile([P, f], mybir.dt.float32)
                nc.scalar.activation(out=wf[:], in_=t1[:], func=act.Identity,
                                     bias=c_nrnd[:], scale=1.0)
                u_t = stream.tile([P, f], mybir.dt.float32)
                nc.vector.scalar_tensor_tensor(
                    out=u_t[:], in0=wf[:], scalar=-SC_POS, in1=v_s[:],
                    op0=op.mult, op1=op.add)
                e_t = stream.tile([P, f], mybir.dt.float32)
                nc.scalar.activation(out=e_t[:], in_=u_t[:], func=act.Exp,
                                     scale=1.0 / SC_S)
                ez = stream.tile([P, f], mybir.dt.float32)
                nc.vector.scalar_tensor_tensor(
                    out=ez[:], in0=v_s[:], scalar=INVALID_THRESH, in1=e_t[:],
                    op0=op.is_lt, op1=op.mult)
                t_t = stream.tile([P, f], mybir.dt.float32)
                nc.vector.tensor_tensor_scan(
                    out=t_t[:], data0=gate[:], data1=ez[:], initial=0.0,
                    op0=op.mult, op1=op.add)
                nc.vector.scalar_tensor_tensor(
                    out=t_t[:], in0=v_s[:], scalar=INVALID_THRESH, in1=t_t[:],
                    op0=op.is_ge, op1=op.add)
                lg = stream.tile([P, f], mybir.dt.float32)
                nc.scalar.activation(out=lg[:], in_=t_t[:], func=act.Ln)
                nc.vector.tensor_reduce(
                    out=lsum_all[:, sup * j:(sup + 1) * j],
                    in_=lg[:].rearrange("p (j n) -> p j n", n=N),
                    axis=mybir.AxisListType.X, op=op.add)

            # ---- epilogue: per-row weighting, partition-level partials
            n_t = singles.tile([P, js], mybir.dt.float32)
            nc.vector.tensor_scalar(out=n_t[:], in0=nm_all[:], scalar1=-1.0,
                                    scalar2=float(N), op0=op.mult, op1=op.add)
            pr0 = singles.tile([P, js], mybir.dt.float32)
            nc.vector.tensor_sub(pr0[:], lsum_all[:], svr_all[:])
            nmx = singles.tile([P, js], mybir.dt.float32)
            nc.vector.tensor_scalar_max(nmx[:], n_t[:], 1.0)
            wrec = singles.tile([P, js], mybir.dt.float32)
            nc.vector.reciprocal(wrec[:], nmx[:])
            use = singles.tile([P, js], mybir.dt.float32)
            nc.vector.tensor_single_scalar(out=use[:], in_=n_t[:], scalar=2.0,
                                           op=op.is_ge)
            w3 = singles.tile([P, js], mybir.dt.float32)
            nc.vector.tensor_tensor(out=w3[:], in0=wrec[:], in1=use[:], op=op.mult)
            pr = singles.tile([P, js], mybir.dt.float32)
            nc.vector.tensor_tensor(out=pr[:], in0=pr0[:], in1=w3[:], op=op.mult)

            out_t = singles.tile([P, 2], mybir.dt.float32)
            nc.vector.tensor_reduce(out=out_t[:, 0:1], in_=pr[:],
                                    axis=mybir.AxisListType.X, op=op.add)
            nc.vector.tensor_reduce(out=out_t[:, 1:2], in_=use[:],
                                    axis=mybir.AxisListType.X, op=op.add)
            nc.sync.dma_start(out=o_d[:], in_=out_t[:])

    nc.finalize()  # run Bacc compile passes (wait splitting, reg alloc)
    return nc


_CACHED = {}


def _get_program():
    if "nc" not in _CACHED:
        _CACHED["nc"] = build_program()
    return _CACHED["nc"]


def _run(scores, ranks, mask, **run_kwargs):
    from concourse.bass_utils import run_bass_kernel_spmd

    nc = _get_program()
    scores = np.ascontiguousarray(np.asarray(scores, dtype=np.float32))
    ranks = np.ascontiguousarray(np.asarray(ranks, dtype=np.int64))
    mask = np.ascontiguousarray(np.asarray(mask))

    in_maps = []
    for c in range(NCORES):
        lo, hi = c * B_CORE, (c + 1) * B_CORE
        in_maps.append({
            "scores": scores[lo:hi],
            "ranks32": ranks[lo:hi].view(np.int32).reshape(B_CORE, 2 * N),
            "mask8": mask[lo:hi].astype(np.uint8),
        })
    res = run_bass_kernel_spmd(nc, in_maps, core_ids=list(range(NCORES)), **run_kwargs)
    partials = np.stack([r["partial"] for r in res.results])  # [8, 128, 2]
    loss_sum = partials[:, :, 0].sum(dtype=np.float64)
    cnt = partials[:, :, 1].sum(dtype=np.float64)
    out = np.float32(loss_sum / max(cnt, 1.0))
    return out, res


def kernel(scores, ranks, mask):
    out, _ = _run(scores, ranks, mask)
    return np.asarray(out, dtype=np.float32)


# revision 18
# speedup vs baseline: 1.2389x; 1.0433x over previous
"""Plackett-Luce listwise loss kernel for Trainium2 (Bass/Tile), 8-core data parallel.

Algorithm (per row of 32 items):
  loss_row = sum_k log(T_k) - sum_i s_i*valid_i, where T_k are the suffix sums
  of exp(s) over items sorted by (rank, position) (stable sort, padded last).
  Final: mean over rows with n>=2 of loss_row/n.

Device strategy: pack key = rank_eff*2^19 + position*2^14 + (s+8)*2^10 into one
fp32 (rank_eff = rank + 64*mask, so padded rows sort last), sort each row's 32
keys DESCENDING with a Batcher odd-even merge network (15 min/max stages on
strided access patterns), then decode the quantized score from the low bits
(s error <= 2^-10, final loss rel err ~1e-7), exp on ScalarE, segmented prefix
scan for the suffix sums, log on ScalarE, and per-row reductions. Each core
reduces its 32768 rows to a [128, 2] partial (weighted loss sum, valid-row
count); the host sums partials and divides.
"""

import sys

for _p in ("/opt/trn_rl_repo", "/root/.axon_site/_ro/trn_rl_repo"):
    if _p not in sys.path:
        sys.path.insert(0, _p)

import numpy as np

P = 128
N = 32
NCORES = 8
B = 262144
B_CORE = B // NCORES  # 32768
J = 32                # row-segments per partition per supertile
F = J * N             # free elements per supertile tile
ROWS_SUP = P * J      # rows per supertile
SUP = B_CORE // ROWS_SUP

# Batcher odd-even merge sort, n=32, descending.
# (k, offset, per-segment pattern [[step,count],...], needs_precopy)
SORT_STAGES = [
    (1, 0, [[2, 16]], False),
    (2, 0, [[4, 8], [1, 2]], False),
    (1, 1, [[4, 8]], True),
    (4, 0, [[8, 4], [1, 4]], False),
    (2, 2, [[8, 4], [1, 2]], True),
    (1, 1, [[8, 4], [2, 3]], True),
    (8, 0, [[16, 2], [1, 8]], False),
    (4, 4, [[16, 2], [1, 4]], True),
    (2, 2, [[16, 2], [4, 3], [1, 2]], True),
    (1, 1, [[16, 2], [2, 7]], True),
    (16, 0, [[1, 16]], False),
    (8, 8, [[1, 8]], True),
    (4, 4, [[8, 3], [1, 4]], True),
    (2, 2, [[4, 7], [1, 2]], True),
    (1, 1, [[2, 15]], True),
]

SC_POS = float(2 ** 14)   # position scale in the packed key
SC_RANK = float(2 ** 19)  # rank scale
SC_S = float(2 ** 10)     # score scale
MASK_BUMP = float(2 ** 25)  # added to the key of padded items
# Valid keys < 2^24 (rank<32); padded keys >= 2^25 - 2^13. Threshold between:
INVALID_THRESH = float(2 ** 24 + 2 ** 23)
RND = float(2 ** 23)      # fp32 round-to-nearest-integer magic constant

# Supertiles whose sort network runs on GPSIMD instead of DVE. Empty: plain
# TensorTensor is not a legal Pool-engine opcode on NeuronCore v3.
GPSIMD_SORT_SUPS = ()


def _pattern_ap(bass_mod, tile_ap, off, dims, j):
    """AP over a [P, j*32] tile selecting `dims` within each 32-item segment."""
    base = tile_ap
    pdim = base.ap[0]
    if dims[0][0] * dims[0][1] == N:
        free = [[dims[0][0], dims[0][1] * j]] + [list(d) for d in dims[1:]]
    else:
        free = [[N, j]] + [list(d) for d in dims]
    return bass_mod.AP(tensor=base.tensor, offset=base.offset + off, ap=[list(pdim)] + free)


def build_program(b_core=B_CORE, j=J):
    import concourse.bass as bass
    import concourse.bacc as bacc
    import concourse.tile as tile
    from concourse import mybir

    f = j * N
    rows_sup = P * j
    sup_count = b_core // rows_sup
    assert b_core % rows_sup == 0

    # Bacc (not raw Bass): its compile() runs generate_event_semaphores, which
    # splits multi-sem waits that TRN2 compute instructions can't encode.
    nc = bacc.Bacc("TRN2")
    s_d = nc.dram_tensor("scores", [b_core, N], mybir.dt.float32, kind="ExternalInput")
    r_d = nc.dram_tensor("ranks32", [b_core, 2 * N], mybir.dt.int32, kind="ExternalInput")
    m_d = nc.dram_tensor("mask8", [b_core, N], mybir.dt.uint8, kind="ExternalInput")
    o_d = nc.dram_tensor("partial", [P, 2], mybir.dt.float32, kind="ExternalOutput")

    op = mybir.AluOpType
    act = mybir.ActivationFunctionType

    with tile.TileContext(nc) as tc:
        with (
            tc.tile_pool(name="singles", bufs=1) as singles,
            tc.tile_pool(name="stream", bufs=2) as stream,
            tc.tile_pool(name="deep", bufs=3) as deep,
        ):
            # constants
            iota14 = singles.tile([P, f], mybir.dt.int32)
            nc.gpsimd.iota(iota14[:], pattern=[[0, j], [int(SC_POS), N]], base=0,
                           channel_multiplier=0)
            gate = singles.tile([P, f], mybir.dt.float32)
            nc.vector.memset(gate[:], 1.0)
            g3 = gate[:].rearrange("p (j n) -> p j n", n=N)
            nc.vector.memset(g3[:, :, 0:1], 0.0)
            c_rnd = singles.tile([P, 1], mybir.dt.float32)
            nc.vector.memset(c_rnd[:], RND)
            c_nrnd = singles.tile([P, 1], mybir.dt.float32)
            nc.vector.memset(c_nrnd[:], -RND)


            # per-row stats accumulated across supertiles
            js = j * sup_count
            lsum_all = singles.tile([P, js], mybir.dt.float32)
            svr_all = singles.tile([P, js], mybir.dt.float32)
            nm_all = singles.tile([P, js], mybir.dt.float32)

            for sup in range(sup_count):
                r0 = sup * rows_sup
                s_t = deep.tile([P, f], mybir.dt.float32)
                nc.sync.dma_start(
                    out=s_t[:],
                    in_=s_d[r0:r0 + rows_sup, :].rearrange("(p j) n -> p (j n)", p=P))
                r_t = deep.tile([P, 2 * f], mybir.dt.int32)
                nc.sync.dma_start(
                    out=r_t[:],
                    in_=r_d[r0:r0 + rows_sup, :].rearrange("(p j) n -> p (j n)", p=P))
                m_t = deep.tile([P, f], mybir.dt.uint8)
                nc.sync.dma_start(
                    out=m_t[:],
                    in_=m_d[r0:r0 + rows_sup, :].rearrange("(p j) n -> p (j n)", p=P))

                # ---- pack V = rank*2^19 + mask*2^25 + i*2^14 + s*2^10
                # chained STT ops on DVE (ACT can't: its sync struct supports a
                # single wait command, so ACT must not read DMA tiles directly)
                r_lo = r_t[:].rearrange("p (f two) -> p f two", two=2)[:, :, 0]
                q_t = stream.tile([P, f], mybir.dt.float32)
                nc.vector.scalar_tensor_tensor(
                    out=q_t[:], in0=r_lo, scalar=SC_RANK, in1=iota14[:],
                    op0=op.mult, op1=op.add)
                w2 = stream.tile([P, f], mybir.dt.float32)
                nc.vector.scalar_tensor_tensor(
                    out=w2[:], in0=m_t[:], scalar=MASK_BUMP, in1=q_t[:],
                    op0=op.mult, op1=op.add)
                v_a = deep.tile([P, f], mybir.dt.float32)
                nc.vector.scalar_tensor_tensor(
                    out=v_a[:], in0=s_t[:], scalar=SC_S, in1=w2[:],
                    op0=op.mult, op1=op.add)

                # ---- per-row masked score sum and mask count (pre-sort)
                sm = stream.tile([P, f], mybir.dt.float32)
                nc.vector.scalar_tensor_tensor(
                    out=sm[:], in0=m_t[:], scalar=0.0, in1=s_t[:],
                    op0=op.is_equal, op1=op.mult)
                nc.vector.tensor_reduce(
                    out=svr_all[:, sup * j:(sup + 1) * j],
                    in_=sm[:].rearrange("p (j n) -> p j n", n=N),
                    axis=mybir.AxisListType.X, op=op.add)
                nc.vector.tensor_reduce(
                    out=nm_all[:, sup * j:(sup + 1) * j],
                    in_=m_t[:].rearrange("p (j n) -> p j n", n=N),
                    axis=mybir.AxisListType.X, op=op.add)

                # ---- Batcher descending sort.
                # Fully-paired stages ping-pong v_a <-> v_b (2 DVE ops).
                # Stages with unpaired lanes run in place: max into a scratch
                # tile, min in place (safe: DVE writes lag reads within an op),
                # then ACT copies scratch back into the low lanes — so the
                # unpaired lanes are never touched and no full-tile copy runs.
                v_b = deep.tile([P, f], mybir.dt.float32)
                scratch = deep.tile([P, f // 2], mybir.dt.float32)
                cur, oth = v_a, v_b
                for (k, off, dims, precopy) in SORT_STAGES:
                    lo_i = _pattern_ap(bass, cur[:], off, dims, j)
                    hi_i = _pattern_ap(bass, cur[:], off + k, dims, j)
                    if precopy:
                        npair = j
                        for d in dims:
                            npair *= d[1]
                        sc = scratch[:, 0:npair]
                        nc.vector.tensor_tensor(out=sc, in0=lo_i, in1=hi_i, op=op.max)
                        nc.vector.tensor_tensor(out=hi_i, in0=lo_i, in1=hi_i, op=op.min)
                        nc.scalar.copy(out=lo_i, in_=sc)
                    else:
                        lo_o = _pattern_ap(bass, oth[:], off, dims, j)
                        hi_o = _pattern_ap(bass, oth[:], off + k, dims, j)
                        nc.vector.tensor_tensor(out=lo_o, in0=lo_i, in1=hi_i, op=op.max)
                        nc.vector.tensor_tensor(out=hi_o, in0=lo_i, in1=hi_i, op=op.min)
                        cur, oth = oth, cur
                v_s = cur

                # ---- decode: u = V mod 2^14 (centered, in (-2^13, 2^13)) via
                # the +2^23 round-to-nearest trick (no mod/convert ISA needed);
                # the two single-src affine steps ride on the idle ACT engine
                t1 = stream.tile([P, f], mybir.dt.float32)
                nc.scalar.activation(out=t1[:], in_=v_s[:], func=act.Identity,
                                     bias=c_rnd[:], scale=1.0 / SC_POS)
                wf = stream.tile([P, f], mybir.dt.float32)
                nc.scalar.activation(out=wf[:], in_=t1[:], func=act.Identity,
                                     bias=c_nrnd[:], scale=1.0)
                u_t = stream.tile([P, f], mybir.dt.float32)
                nc.vector.scalar_tensor_tensor(
                    out=u_t[:], in0=wf[:], scalar=-SC_POS, in1=v_s[:],
                    op0=op.mult, op1=op.add)
                e_t = stream.tile([P, f], mybir.dt.float32)
                nc.scalar.activation(out=e_t[:], in_=u_t[:], func=act.Exp,
                                     scale=1.0 / SC_S)
                ez = stream.tile([P, f], mybir.dt.float32)
                nc.vector.scalar_tensor_tensor(
                    out=ez[:], in0=v_s[:], scalar=INVALID_THRESH, in1=e_t[:],
                    op0=op.is_lt, op1=op.mult)
                t_t = stream.tile([P, f], mybir.dt.float32)
                nc.vector.tensor_tensor_scan(
                    out=t_t[:], data0=gate[:], data1=ez[:], initial=0.0,
                    op0=op.mult, op1=op.add)
                nc.vector.scalar_tensor_tensor(
                    out=t_t[:], in0=v_s[:], scalar=INVALID_THRESH, in1=t_t[:],
                    op0=op.is_ge, op1=op.add)
                lg = stream.tile([P, f], mybir.dt.float32)
                nc.scalar.activation(out=lg[:], in_=t_t[:], func=act.Ln)
                nc.vector.tensor_reduce(
                    out=lsum_all[:, sup * j:(sup + 1) * j],
                    in_=lg[:].rearrange("p (j n) -> p j n", n=N),
                    axis=mybir.AxisListType.X, op=op.add)

            # ---- epilogue: per-row weighting, partition-level partials
            n_t = singles.tile([P, js], mybir.dt.float32)
            nc.vector.tensor_scalar(out=n_t[:], in0=nm_all[:], scalar1=-1.0,
                                    scalar2=float(N), op0=op.mult, op1=op.add)
            pr0 = singles.tile([P, js], mybir.dt.float32)
            nc.vector.tensor_sub(pr0[:], lsum_all[:], svr_all[:])
            nmx = singles.tile([P, js], mybir.dt.float32)
            nc.vector.tensor_scalar_max(nmx[:], n_t[:], 1.0)
            wrec = singles.tile([P, js], mybir.dt.float32)
            nc.vector.reciprocal(wrec[:], nmx[:])
            use = singles.tile([P, js], mybir.dt.float32)
            nc.vector.tensor_single_scalar(out=use[:], in_=n_t[:], scalar=2.0,
                                           op=op.is_ge)
            w3 = singles.tile([P, js], mybir.dt.float32)
            nc.vector.tensor_tensor(out=w3[:], in0=wrec[:], in1=use[:], op=op.mult)
            pr = singles.tile([P, js], mybir.dt.float32)
            nc.vector.tensor_tensor(out=pr[:], in0=pr0[:], in1=w3[:], op=op.mult)

            out_t = singles.tile([P, 2], mybir.dt.float32)
            nc.vector.tensor_reduce(out=out_t[:, 0:1], in_=pr[:],
                                    axis=mybir.AxisListType.X, op=op.add)
            nc.vector.tensor_reduce(out=out_t[:, 1:2], in_=use[:],
                                    axis=mybir.AxisListType.X, op=op.add)
            nc.sync.dma_start(out=o_d[:], in_=out_t[:])

    nc.finalize()  # run Bacc compile passes (wait splitting, reg alloc)
    return nc


_CACHED = {}


def _get_program():
    if "nc" not in _CACHED:
        _CACHED["nc"] = build_program()
    return _CACHED["nc"]


def _run(scores, ranks, mask, **run_kwargs):
    from concourse.bass_utils import run_bass_kernel_spmd

    nc = _get_program()
    scores = np.ascontiguousarray(np.asarray(scores, dtype=np.float32))
    ranks = np.ascontiguousarray(np.asarray(ranks, dtype=np.int64))
    mask = np.ascontiguousarray(np.asarray(mask))

    in_maps = []
    for c in range(NCORES):
        lo, hi = c * B_CORE, (c + 1) * B_CORE
        in_maps.append({
            "scores": scores[lo:hi],
            "ranks32": ranks[lo:hi].view(np.int32).reshape(B_CORE, 2 * N),
            "mask8": mask[lo:hi].astype(np.uint8),
        })
    res = run_bass_kernel_spmd(nc, in_maps, core_ids=list(range(NCORES)), **run_kwargs)
    partials = np.stack([r["partial"] for r in res.results])  # [8, 128, 2]
    loss_sum = partials[:, :, 0].sum(dtype=np.float64)
    cnt = partials[:, :, 1].sum(dtype=np.float64)
    out = np.float32(loss_sum / max(cnt, 1.0))
    return out, res


def kernel(scores, ranks, mask):
    out, _ = _run(scores, ranks, mask)
    return np.asarray(out, dtype=np.float32)
